# revision 8
# baseline (speedup 1.0000x reference)
"""Trainium2 Bass kernel for nn_CustomMoETransformer (8-core SPMD).

Sharding: attention head-sharded (2 heads/core), MoE expert-parallel
(1 expert/core, dense over tokens). Activation spine transposed [H, T].

v2 layout: x pre-scaled by 1/8 on host (xs = x/8, eps folded to eps/64,
0.125 folded into wo), so each core's MoE output contribution carries
hs = h/8 and the final ReduceScatter sum reconstructs h + moe directly.
Attention reduce: bf16 ReduceScatter + AllGather (cheaper than AllReduce).
Final reduce: bf16 ReduceScatter straight into the outT shard; host
assembles the 8 shards. MoE weights bf16, loaded once (it-outer loop,
full-T g buffer). No debug outputs, no phase-7 residual pass.
"""
import sys
sys.path.insert(0, '/opt/trn_rl_repo')
import numpy as np
import ml_dtypes

import concourse.bacc as bacc
import concourse.mybir as mybir
import concourse.tile as tile
from concourse.bass_utils import run_bass_kernel_spmd

NC = 8
H = 1024
T = 2048
S = 1024
I = 2048
KC = 8
NIT = 16
NT = 4
EPS = 1e-6
F32 = mybir.dt.float32
F32R = mybir.dt.float32r
BF16 = mybir.dt.bfloat16
ADD = mybir.AluOpType.add
MULT = mybir.AluOpType.mult
BYP = mybir.AluOpType.bypass
AX = mybir.AxisListType.X
AF = mybir.ActivationFunctionType

_CACHE = {}


def build_nc():
    nc = bacc.Bacc()
    def inp(name, shape, dt):
        return nc.declare_dram_parameter(name, list(shape), dt, isOutput=False)

    xT_d   = inp("xT",   (H, T), F32)     # x / 8, transposed
    wq_d   = inp("wq_c", (H, 128), F32)   # anw + 0.125 folded
    wk_d   = inp("wk_c", (H, 128), F32)   # anw folded
    wv_d   = inp("wv_c", (H, 128), F32)   # anw folded
    wo_d   = inp("wo_c", (128, H), F32)   # 0.125 folded
    rw_d   = inp("rw",   (H, 8), F32)     # fnw folded
    w1_d   = inp("w1_c", (H, I), BF16)    # fnw folded
    w3_d   = inp("w3_c", (H, I), BF16)    # fnw folded
    w2_d   = inp("w2_c", (I, H), BF16)
    cos_d  = inp("cos64", (64, T), F32)
    sin_d  = inp("sin64", (64, T), F32)
    msk_d  = inp("masks", (4, 128, 512), F32)
    eye_d  = inp("eye",  (128, 128), F32)
    s64_d  = inp("S64",  (64, 64), F32)
    cvr_d  = inp("cvecr", (128, 2), F32)
    onr_d  = inp("onesr", (1, 128), F32)
    epc_d  = inp("epsc",  (1, 1), F32)    # EPS / 64
    sel_d  = inp("sel8", (8, 1), F32)
    outT_d = nc.declare_dram_parameter("outT", [NT * 128, 512], BF16, isOutput=True)

    RG = [list(range(NC))]

    # DRAM scratch (raw tensors; AG outputs Shared for fast collectives)
    arin  = [nc.dram_tensor(f"ari{j}", [H, 512], BF16) for j in range(NT)]
    rsb   = [nc.dram_tensor(f"rsb{j}", [128, 512], BF16) for j in range(NT)]
    arout = [nc.dram_tensor(f"aro{j}", [H, 512], BF16, addr_space="Shared")
             for j in range(NT)]
    min_d = [nc.dram_tensor(f"mi{j}", [H, 512], BF16) for j in range(NT)]
    rso   = [nc.dram_tensor(f"rso{j}", [128, 512], BF16) for j in range(NT)]

    with tile.TileContext(nc) as tc, nc.allow_low_precision(reason="fp32r/bf16 rounding intentional"):
      with tc.tile_pool(name="pc", bufs=1) as pc:
        # ---- constants ----
        cvr = pc.tile([128, 2], F32R, tag="cvr", name="cvr"); nc.gpsimd.dma_start(out=cvr[:], in_=cvr_d[:, :])
        onr = pc.tile([1, 128], F32R, tag="onr", name="onr"); nc.gpsimd.dma_start(out=onr[:], in_=onr_d[:, :])
        eps1 = pc.tile([1, 1], F32, tag="eps1", name="eps1"); nc.sync.dma_start(out=eps1[:], in_=epc_d[:, :])
        ones128 = cvr[:, 0:1]
        oH      = cvr[:, 1:2]
        ones1a  = onr[:, 0:128]
        ones1b  = onr[:, 0:64]
        sel_sb  = pc.tile([8, 1],  F32R, tag="sel", name="sel");  nc.gpsimd.dma_start(out=sel_sb[:], in_=sel_d[:, :])
        s64_sb  = pc.tile([64, 64], F32R, tag="s64", name="s64"); nc.gpsimd.dma_start(out=s64_sb[:], in_=s64_d[:, :])
        # xs = x/8 resident; phase 5 overwrites with hs = h/8
        xt = [pc.tile([128, T], F32R, tag=f"x{k}", name=f"x{k}") for k in range(KC)]

        # ============ attention span ============
        with (
          tc.tile_pool(name="pqk", bufs=1) as pqk,
          tc.tile_pool(name="pqs", bufs=2) as pqs,
        ):
          cos_sb = pqk.tile([64, T], F32, tag="cos", name="cos"); nc.sync.dma_start(out=cos_sb[:], in_=cos_d[:, :])
          sin_sb = pqk.tile([64, T], F32, tag="sin", name="sin"); nc.sync.dma_start(out=sin_sb[:], in_=sin_d[:, :])
          msk_sb = pqk.tile([128, 4, 512], F32, tag="msk", name="msk")
          nc.sync.dma_start(out=msk_sb[:], in_=msk_d[:, :, :].rearrange("v p q -> p v q"))
          woa_sb = pqk.tile([64, H], F32R, tag="woa", name="woa"); nc.gpsimd.dma_start(out=woa_sb[:], in_=wo_d[0:64, :])
          wob_sb = pqk.tile([64, H], F32R, tag="wob", name="wob"); nc.gpsimd.dma_start(out=wob_sb[:], in_=wo_d[64:128, :])
          wq_sb = pqk.tile([128, KC, 2, 64], F32R, tag="wq", name="wq")
          nc.gpsimd.dma_start(out=wq_sb[:], in_=wq_d[:, :].rearrange("(k p) (hp d) -> p k hp d", p=128, hp=2))
          wk_sb = pqk.tile([128, KC, 2, 64], F32R, tag="wk", name="wk")
          nc.gpsimd.dma_start(out=wk_sb[:], in_=wk_d[:, :].rearrange("(k p) (hp d) -> p k hp d", p=128, hp=2))
          wv_sb = pqk.tile([128, KC, 128], F32R, tag="wv", name="wv")
          nc.gpsimd.dma_start(out=wv_sb[:], in_=wv_d[:, :].rearrange("(k p) m -> p k m", p=128))

          q2 = pqk.tile([64, 2 * T], F32R, tag="q2", name="q2")
          k2 = pqk.tile([64, 2 * T], F32R, tag="k2", name="k2")
          vn = pqk.tile([128, 16, 128], F32R, tag="vn", name="vn")
          inv1 = pqk.tile([1, T], F32R, tag="inv1", name="inv1")
          inv1f = pqk.tile([1, T], F32, tag="inv1f", name="inv1f")
          one11f = pqk.tile([1, 1], F32, tag="one11f", name="one11f"); nc.vector.memset(one11f[:], 1.0)
          invcol = pqk.tile([128, 16], F32, tag="invcol", name="invcol")

          # ---- phase 1: load xs, rms stats ----
          with (
            tc.tile_pool(name="p1s", bufs=2) as p1s,
            tc.tile_pool(name="ps1", bufs=1, space="PSUM") as ps1,
            tc.tile_pool(name="ps1b", bufs=2, space="PSUM") as ps1b,
          ):
            ssq = [ps1.tile([1, 512], F32, tag=f"ssq{j}", name=f"ssq{j}") for j in range(NT)]
            for k in range(KC):
                nc.gpsimd.dma_start(out=xt[k][:], in_=xT_d[128*k:128*(k+1), :])
                for j in range(NT):
                    sq = p1s.tile([128, 512], F32R, tag="sq", name="sq")
                    nc.scalar.activation(sq[:], xt[k][:, 512*j:512*(j+1)], AF.Square)
                    nc.tensor.matmul(ssq[j][:], oH, sq[:], start=(k == 0), stop=(k == KC-1))
            for j in range(NT):
                rms1 = p1s.tile([1, 512], F32, tag="rms1", name="rms1")
                nc.scalar.activation(rms1[:], ssq[j][:], AF.Sqrt, bias=eps1[:])
                nc.vector.reciprocal(inv1f[:, 512*j:512*(j+1)], rms1[:])
                nc.scalar.copy(out=inv1[:, 512*j:512*(j+1)], in_=inv1f[:, 512*j:512*(j+1)])
            # invcol[t%128 partition, tt] = inv1[t] via PE transpose
            for tt in range(16):
                icp = ps1b.tile([128, 1], F32, tag="icp", name="icp")
                nc.tensor.transpose(icp[:], inv1f[:, 128*tt:128*(tt+1)], one11f[:])
                nc.scalar.copy(out=invcol[:, tt:tt+1], in_=icp[:])

          # ---- phase 2: QKV (raw) + inv scaling + RoPE ----
          with (
            tc.tile_pool(name="p2", bufs=1) as p2,
            tc.tile_pool(name="ps2", bufs=2, space="PSUM") as ps2,
          ):
            q2r = p2.tile([64, 2 * T], F32R, tag="q2r", name="q2r")
            k2r = p2.tile([64, 2 * T], F32R, tag="k2r", name="k2r")
            for hp in range(2):
              for j in range(NT):
                qp = ps2.tile([64, 512], F32, tag="qp", name="qp")
                kp = ps2.tile([64, 512], F32, tag="kp", name="kp")
                for k in range(KC):
                    nc.tensor.matmul(qp[:], wq_sb[:, k, hp, :], xt[k][:, 512*j:512*(j+1)],
                                     start=(k == 0), stop=(k == KC-1))
                for k in range(KC):
                    nc.tensor.matmul(kp[:], wk_sb[:, k, hp, :], xt[k][:, 512*j:512*(j+1)],
                                     start=(k == 0), stop=(k == KC-1))
                c0 = hp * T + 512 * j
                nc.scalar.copy(out=q2r[:, c0:c0+512], in_=qp[:])
                nc.scalar.copy(out=k2r[:, c0:c0+512], in_=kp[:])
            for tt in range(16):
                vp = ps2.tile([128, 128], F32, tag="vp", name="vp")
                for k in range(KC):
                    nc.tensor.matmul(vp[:], xt[k][:, 128*tt:128*(tt+1)], wv_sb[:, k, :],
                                     start=(k == 0), stop=(k == KC-1))
                nc.vector.tensor_scalar(out=vn[:, tt, :], in0=vp[:],
                                        scalar1=invcol[:, tt:tt+1], scalar2=None, op0=MULT)
            # RoPE + per-token inv: dst = (src*cos + (S64.T@src)*sin) * inv
            for rsrc, dst in ((q2r, q2), (k2r, k2)):
              for n in range(8):
                sl = slice(512*n, 512*(n+1))
                tsl = slice((512*n) % T, (512*n) % T + 512)
                sw = ps2.tile([64, 512], F32, tag="qp", name="qp")
                nc.tensor.matmul(sw[:], s64_sb[:], rsrc[:, sl], start=True, stop=True)
                nc.vector.tensor_tensor(out=dst[:, sl], in0=rsrc[:, sl], in1=cos_sb[:, tsl], op=MULT)
                tb = pqs.tile([64, 512], F32, tag="rb", name="rb")
                nc.vector.tensor_tensor(out=tb[:], in0=sw[:], in1=sin_sb[:, tsl], op=MULT)
                nc.vector.tensor_tensor(out=dst[:, sl], in0=dst[:, sl], in1=tb[:], op=ADD)
                ib = ps2.tile([64, 512], F32, tag="kp", name="kp")
                nc.tensor.matmul(ib[:], ones1b, inv1[:, tsl], start=True, stop=True)
                nc.vector.tensor_tensor(out=dst[:, sl], in0=dst[:, sl], in1=ib[:], op=MULT)

          # ---- phase 3+4: attention + wo + chunked RS/AG ----
          with (
            tc.tile_pool(name="p3", bufs=3) as p3,
            tc.tile_pool(name="ps3", bufs=2, space="PSUM") as ps3,
            tc.tile_pool(name="ps3b", bufs=1, space="PSUM") as ps3b,
            tc.tile_pool(name="ps4", bufs=2, space="PSUM") as ps4,
          ):
            for b in range(2):
              for qt in range(2):
                j = 2*b + qt
                oT_loc = []
                for hp in range(2):
                  base = hp * T + b * S
                  qsl = slice(base + 512*qt, base + 512*(qt+1))
                  kts = list(range(4*qt + 4))
                  sump = ps3.tile([1, 512], F32, tag="sump", name="sump", bufs=1)
                  op_ = ps3.tile([64, 512], F32, tag="op", name="op")
                  for i, kt in enumerate(kts):
                    scp = ps3.tile([128, 512], F32, tag="scp", name="scp")
                    nc.tensor.matmul(scp[:], k2[:, base + 128*kt: base + 128*(kt+1)],
                                     q2[:, qsl], start=True, stop=True)
                    off = 512*qt - 128*kt
                    if off < 127:
                        vidx = (-off) // 128
                        nc.vector.tensor_tensor(out=scp[:], in0=scp[:],
                                                in1=msk_sb[:, vidx, :], op=ADD)
                    at = p3.tile([128, 512], F32R, tag="at", name="at")
                    nc.scalar.activation(at[:], scp[:], AF.Exp)
                    nc.tensor.matmul(sump[:], ones128, at[:],
                                     start=(i == 0), stop=(i == len(kts)-1))
                    nc.tensor.matmul(op_[:], vn[:, b*8 + kt, 64*hp:64*(hp+1)], at[:],
                                     start=(i == 0), stop=(i == len(kts)-1))
                  rec = p3.tile([1, 512], F32R, tag="rec", name="rec")
                  nc.vector.reciprocal(rec[:], sump[:])
                  bcr = ps3b.tile([64, 512], F32, tag="bcr", name="bcr")
                  nc.tensor.matmul(bcr[:], ones1b, rec[:], start=True, stop=True)
                  bcs = p3.tile([64, 512], F32, tag="bcs", name="bcs")
                  nc.scalar.copy(out=bcs[:], in_=bcr[:])
                  ot = p3.tile([64, 512], F32R, tag="ot", name="ot")
                  nc.vector.tensor_tensor(out=ot[:], in0=op_[:], in1=bcs[:], op=MULT)
                  oT_loc.append(ot)
                for m in range(KC):
                  yp = ps4.tile([128, 512], F32, tag="yp", name="yp")
                  for hp, wsb in ((0, woa_sb), (1, wob_sb)):
                      nc.tensor.matmul(yp[:], wsb[:, 128*m:128*(m+1)], oT_loc[hp][:],
                                       start=(hp == 0), stop=(hp == 1))
                  yw = p3.tile([128, 512], BF16, tag="yw", name="yw")
                  nc.scalar.copy(out=yw[:], in_=yp[:])
                  nc.sync.dma_start(out=arin[j][128*m:128*(m+1), :], in_=yw[:])
                nc.gpsimd.collective_compute(
                    "ReduceScatter", ADD, ins=[arin[j][:, :].opt()],
                    outs=[rsb[j][:, :].opt()], replica_groups=RG)
                nc.gpsimd.collective_compute(
                    "AllGather", BYP, ins=[rsb[j][:, :].opt()],
                    outs=[arout[j][:, :].opt()], replica_groups=RG)

        # ============ FFN span ============
        with tc.tile_pool(name="pp", bufs=1) as pp:
          xn2 = [pp.tile([128, T], BF16, tag=f"xn2_{k}", name=f"xn2_{k}") for k in range(KC)]
          combT = pp.tile([8, T], F32R, tag="combT", name="combT")
          bcg_sb = pp.tile([128, T], F32, tag="bcg", name="bcg")
          rw_sb = pp.tile([128, KC, 8], F32R, tag="rw", name="rw")
          nc.gpsimd.dma_start(out=rw_sb[:], in_=rw_d[:, :].rearrange("(k p) e -> p k e", p=128))
          eye_sb = pp.tile([128, 128], F32, tag="eye", name="eye")
          nc.sync.dma_start(out=eye_sb[:], in_=eye_d[:, :])
          one11f2 = pp.tile([1, 1], F32, tag="one11f2", name="one11f2"); nc.vector.memset(one11f2[:], 1.0)

          # ---- phase 5: residual (hs = xs + attn/8) + rmsnorm2 + router ----
          with (
            tc.tile_pool(name="p5s", bufs=2) as p5s,
            tc.tile_pool(name="ps5", bufs=1, space="PSUM") as ps5,
            tc.tile_pool(name="ps5s", bufs=1, space="PSUM") as ps5s,
          ):
            for j in range(NT):
              jsl = slice(512*j, 512*(j+1))
              ssq2 = ps5.tile([1, 512], F32, tag="ssq2", name="ssq2")
              for k in range(KC):
                aro = p5s.tile([128, 512], BF16, tag="aro", name="aro")
                nc.sync.dma_start(out=aro[:], in_=arout[j][128*k:128*(k+1), :])
                nc.vector.tensor_tensor(out=xt[k][:, jsl], in0=xt[k][:, jsl], in1=aro[:], op=ADD)
                sq2 = p5s.tile([128, 512], F32R, tag="sq2", name="sq2")
                nc.scalar.activation(sq2[:], xt[k][:, jsl], AF.Square)
                nc.tensor.matmul(ssq2[:], oH, sq2[:], start=(k == 0), stop=(k == KC-1))
              rms2 = p5s.tile([1, 512], F32, tag="rms2", name="rms2")
              nc.scalar.activation(rms2[:], ssq2[:], AF.Sqrt, bias=eps1[:])
              inv2f = p5s.tile([1, 512], F32, tag="inv2f", name="inv2f")
              nc.vector.reciprocal(inv2f[:], rms2[:])
              inv2 = p5s.tile([1, 512], F32R, tag="inv2", name="inv2")
              nc.scalar.copy(out=inv2[:], in_=inv2f[:])
              bc2p = ps5s.tile([128, 512], F32, tag="smallp", name="smallp")
              nc.tensor.matmul(bc2p[:], ones1a, inv2[:], start=True, stop=True)
              bc2 = p5s.tile([128, 512], F32, tag="bc2", name="bc2")
              nc.scalar.copy(out=bc2[:], in_=bc2p[:])
              for k in range(KC):
                nc.vector.tensor_tensor(out=xn2[k][:, jsl], in0=xt[k][:, jsl],
                                        in1=bc2[:], op=MULT)
              # router for this block
              for tl in range(4):
                tt = 4*j + tl
                tsl = slice(512*j + 128*tl, 512*j + 128*(tl+1))
                lgp = ps5.tile([128, 8], F32, tag="lgp", name="lgp")
                for k in range(KC):
                    nc.tensor.matmul(lgp[:], xt[k][:, tsl], rw_sb[:, k, :],
                                     start=(k == 0), stop=(k == KC-1))
                i2p = ps5s.tile([128, 1], F32, tag="i2p", name="i2p")
                nc.tensor.transpose(i2p[:], inv2f[:, 128*tl:128*(tl+1)], one11f2[:])
                i2c = p5s.tile([128, 1], F32, tag="i2c", name="i2c")
                nc.scalar.copy(out=i2c[:], in_=i2p[:])
                lgs = p5s.tile([128, 8], F32, tag="lgs", name="lgs")
                nc.vector.tensor_scalar(out=lgs[:], in0=lgp[:], scalar1=i2c[:],
                                        scalar2=None, op0=MULT)
                r = p5s.tile([128, 48], F32, tag="rsc", name="rsc")
                el  = r[:, 0:8]; is1 = r[:, 8:16]; t1 = r[:, 16:24]; mk = r[:, 24:32]
                is2 = r[:, 32:40]; cb = r[:, 40:48]
                s = p5s.tile([128, 8], F32, tag="rss", name="rss")
                m1 = s[:, 0:1]; m2 = s[:, 1:2]; dn = s[:, 2:3]; rc = s[:, 3:4]
                nc.scalar.activation(el, lgs[:], AF.Exp)
                nc.vector.reduce_max(m1, el, axis=AX)
                nc.vector.tensor_scalar(out=is1, in0=el, scalar1=m1, scalar2=None,
                                        op0=mybir.AluOpType.is_equal)
                nc.vector.tensor_tensor(out=t1, in0=el, in1=is1, op=MULT)
                nc.vector.tensor_tensor(out=mk, in0=el, in1=t1, op=mybir.AluOpType.subtract)
                nc.vector.reduce_max(m2, mk, axis=AX)
                nc.vector.tensor_scalar(out=is2, in0=mk, scalar1=m2, scalar2=None,
                                        op0=mybir.AluOpType.is_equal)
                nc.vector.tensor_tensor(out=is1, in0=is1, in1=is2, op=ADD)
                nc.vector.tensor_tensor(out=t1, in0=el, in1=is1, op=MULT)
                nc.vector.tensor_tensor(out=dn, in0=m1, in1=m2, op=ADD)
                nc.vector.reciprocal(rc, dn)
                nc.vector.tensor_scalar(out=cb, in0=t1, scalar1=rc, scalar2=None, op0=MULT)
                ctp = ps5s.tile([8, 128], F32, tag="ctp", name="ctp")
                nc.tensor.transpose(ctp[:], cb, eye_sb[:])
                nc.scalar.copy(out=combT[:, 128*tt:128*(tt+1)], in_=ctp[:])
              rEp = ps5s.tile([1, 512], F32, tag="smallp", name="smallp")
              nc.tensor.matmul(rEp[:], sel_sb[:], combT[:, jsl], start=True, stop=True)
              rE = p5s.tile([1, 512], F32R, tag="rE", name="rE")
              nc.scalar.copy(out=rE[:], in_=rEp[:])
              bgp = ps5s.tile([128, 512], F32, tag="smallp", name="smallp")
              nc.tensor.matmul(bgp[:], ones1a, rE[:], start=True, stop=True)
              nc.scalar.copy(out=bcg_sb[:, jsl], in_=bgp[:])

          # ---- phase 6: MoE expert (dense, it-outer, weights loaded once) ----
          with (
            tc.tile_pool(name="p6", bufs=1) as p6,
            tc.tile_pool(name="p6s", bufs=2) as p6s,
            tc.tile_pool(name="ps6", bufs=2, space="PSUM") as ps6,
            tc.tile_pool(name="ps6b", bufs=2, space="PSUM") as ps6b,
          ):
            g_sb = p6.tile([128, NIT, T], BF16, tag="g", name="g")
            for it in range(NIT):
              w1t = p6s.tile([128, KC, 128], BF16, tag="w1t", name="w1t")
              nc.sync.dma_start(out=w1t[:], in_=w1_d[:, 128*it:128*(it+1)]
                                .rearrange("(k p) m -> p k m", p=128))
              w3t = p6s.tile([128, KC, 128], BF16, tag="w3t", name="w3t")
              nc.scalar.dma_start(out=w3t[:], in_=w3_d[:, 128*it:128*(it+1)]
                                .rearrange("(k p) m -> p k m", p=128))
              for q in range(NT):
                csl = slice(512*q, 512*(q+1))
                h1p = ps6.tile([128, 512], F32, tag="h1p", name="h1p")
                h3p = ps6.tile([128, 512], F32, tag="h3p", name="h3p")
                for k in range(KC):
                    nc.tensor.matmul(h1p[:], w1t[:, k, :], xn2[k][:, csl],
                                     start=(k == 0), stop=(k == KC-1))
                for k in range(KC):
                    nc.tensor.matmul(h3p[:], w3t[:, k, :], xn2[k][:, csl],
                                     start=(k == 0), stop=(k == KC-1))
                sil = p6s.tile([128, 512], F32R, tag="sil", name="sil")
                nc.scalar.activation(sil[:], h1p[:], AF.Silu)
                nc.vector.tensor_tensor(out=g_sb[:, it, csl], in0=sil[:], in1=h3p[:], op=MULT)
            for m in range(KC):
              w2t = p6s.tile([128, NIT, 128], BF16, tag="w2t", name="w2t")
              nc.gpsimd.dma_start(out=w2t[:], in_=w2_d[:, 128*m:128*(m+1)]
                                  .rearrange("(i p) m -> p i m", p=128))
              for q in range(NT):
                csl = slice(512*q, 512*(q+1))
                yep = ps6b.tile([128, 512], F32, tag="yep", name="yep")
                for it in range(NIT):
                    nc.tensor.matmul(yep[:], w2t[:, it, :], g_sb[:, it, csl],
                                     start=(it == 0), stop=(it == NIT-1))
                yg = p6s.tile([128, 512], F32, tag="yg", name="yg")
                nc.vector.tensor_tensor(out=yg[:], in0=yep[:], in1=bcg_sb[:, csl], op=MULT)
                yv = p6s.tile([128, 512], BF16, tag="yv", name="yv")
                nc.vector.tensor_tensor(out=yv[:], in0=yg[:], in1=xt[m][:, csl], op=ADD)
                nc.sync.dma_start(out=min_d[q][128*m:128*(m+1), :], in_=yv[:])
            for j in range(NT):
              nc.gpsimd.collective_compute(
                  "ReduceScatter", ADD, ins=[min_d[j][:, :].opt()],
                  outs=[rso[j][:, :].opt()], replica_groups=RG)
              nc.sync.dma_start(out=outT_d[128*j:128*(j+1), :], in_=rso[j][:, :])

    nc.finalize()
    return nc


def _host_prep(inputs):
    x = np.asarray(inputs['x'], np.float32)
    fc = np.asarray(inputs['freqs_cis'], np.float32)
    anw = np.asarray(inputs['attn_norm_w'], np.float32)
    fnw = np.asarray(inputs['ffn_norm_w'], np.float32)
    xT = np.ascontiguousarray(x.reshape(T, H).T) * 0.125
    pos = (np.arange(T) % S)
    d = np.arange(64)
    cos64 = np.ascontiguousarray(fc[pos[None, :], 2 * (d[:, None] // 2)])
    sin64 = np.ascontiguousarray(fc[pos[None, :], 2 * (d[:, None] // 2) + 1])
    S64 = np.zeros((64, 64), np.float32)
    ii = np.arange(0, 64, 2)
    S64[ii + 1, ii] = -1.0
    S64[ii, ii + 1] = 1.0
    masks = np.zeros((4, 128, 512), np.float32)
    kr = np.arange(128)[:, None]
    qr = np.arange(512)[None, :]
    for v in range(4):
        masks[v] = np.where(kr + 128*v <= qr, 0.0, -1e9).astype(np.float32)
    eye = np.eye(128, dtype=np.float32)
    cvecr = np.zeros((128, 2), np.float32); cvecr[:, 0] = 1.0; cvecr[:, 1] = 1.0/H
    onesr = np.ones((1, 128), np.float32)
    epsc = np.full((1, 1), EPS / 64.0, np.float32)
    wq = np.asarray(inputs['wq'], np.float32) * anw[:, None] * 0.125
    wk = np.asarray(inputs['wk'], np.float32) * anw[:, None]
    wv = np.asarray(inputs['wv'], np.float32) * anw[:, None]
    wo = np.asarray(inputs['wo'], np.float32) * 0.125
    bf = ml_dtypes.bfloat16
    rw = np.asarray(inputs['router_w'], np.float32) * fnw[:, None]
    w1 = (np.asarray(inputs['w1'], np.float32) * fnw[None, :, None]).astype(bf)
    w3 = (np.asarray(inputs['w3'], np.float32) * fnw[None, :, None]).astype(bf)
    w2 = np.asarray(inputs['w2'], np.float32).astype(bf)
    maps = []
    for c in range(NC):
        sel = np.zeros((8, 1), np.float32); sel[c, 0] = 1.0
        maps.append({
            "xT": xT,
            "wq_c": np.ascontiguousarray(wq[:, 128*c:128*(c+1)]),
            "wk_c": np.ascontiguousarray(wk[:, 128*c:128*(c+1)]),
            "wv_c": np.ascontiguousarray(wv[:, 128*c:128*(c+1)]),
            "wo_c": np.ascontiguousarray(wo[128*c:128*(c+1), :]),
            "rw":   rw,
            "w1_c": np.ascontiguousarray(w1[c]),
            "w3_c": np.ascontiguousarray(w3[c]),
            "w2_c": np.ascontiguousarray(w2[c]),
            "cos64": cos64, "sin64": sin64,
            "masks": masks, "eye": eye,
            "S64": S64, "sel8": sel,
            "cvecr": cvecr, "onesr": onesr, "epsc": epsc,
        })
    return maps


def kernel(**inputs):
    if 'nc' not in _CACHE:
        _CACHE['nc'] = build_nc()
    nc = _CACHE['nc']
    maps = _host_prep(inputs)
    res = run_bass_kernel_spmd(nc, maps, list(range(NC)))
    full = np.empty((H, T), np.float32)
    for c in range(NC):
        sh = np.asarray(res.results[c]["outT"], dtype=np.float32).reshape(NT, 128, 512)
        for j in range(NT):
            full[128*c:128*(c+1), 512*j:512*(j+1)] = sh[j]
    return np.ascontiguousarray(full.T).reshape(2, S, H)
